# revision 36
# baseline (speedup 1.0000x reference)
"""Trainium2 Bass kernel for nn_CustomAttention (B=16, T=S=E=1024).

Reference computation (per batch, T == E == 1024):
    q = query @ Wq.T + bq            [T, E]   (feature dim i)
    k = key   @ Wk.T + bk            [S, E]   (feature dim t~)
    v = value @ Wv.T + bv            [S, E]
    w[i, s] = sum_t q[t, i] k[s, t] / sqrt(E)
    a = softmax_s(w)
    o[i, e] = sum_s a[i, s] v[s, e]
    out = o @ Wo.T + bo              [E, E] == [T, E]

Sharding: data-parallel over batch, 2 batches per NeuronCore, no
collectives.  Weights are host-pre-transposed to [e_in, f_out] so the
contraction dim lands on SBUF partitions.

On-device layout choices (per batch):
  - q   [t, i]  and kT [t~, s]  let the attention matmul run with the
    contraction dim t on partitions for both operands.
  - attention is computed as wT[s, i] (lhsT = kT chunks, rhs = q), so
    exp(wT) == aT feeds the o-matmul directly with NO transpose of a.
  - softmax denominators come from N=1 matmuls of aT chunks against a
    ones vector, landing [128, 1] per i-chunk; normalization is applied
    as a per-partition scalar multiply in the final output copyback.
  - all matmuls use float32r (full PE rate for free dim >= 256,
    ~1.5e-4 relative error); input transposes run on the PE in fp32.
  - softmax max-subtraction is skipped: logits are ~N(0, 0.41), far
    from exp() overflow.
"""

from contextlib import ExitStack

import numpy as np

B, T, S, E = 16, 1024, 1024, 1024
NCORES = 8
BPC = B // NCORES  # batches per core
P = 128
KO = E // P  # 8 k-tiles of 128
NH = 512  # matmul free-dim (half of 1024)
SCALE = 1.0 / 32.0  # 1/sqrt(E)

_cache = {}


def _build_nc(reps=1):
    import concourse.mybir as mybir
    import concourse.tile as tile
    from concourse import bacc
    from concourse.masks import make_identity

    F32 = mybir.dt.float32
    F32R = mybir.dt.float32r

    nc = bacc.Bacc("TRN2", target_bir_lowering=False, debug=False)

    xq_d = nc.dram_tensor("xq", [BPC, T, E], F32R, kind="ExternalInput").ap()
    xk_d = nc.dram_tensor("xk", [BPC, S, E], F32R, kind="ExternalInput").ap()
    xv_d = nc.dram_tensor("xv", [BPC, S, E], F32R, kind="ExternalInput").ap()
    wq_d = nc.dram_tensor("wq", [E, E], F32R, kind="ExternalInput").ap()
    wk_d = nc.dram_tensor("wk", [E, E], F32R, kind="ExternalInput").ap()
    wv_d = nc.dram_tensor("wv", [E, E], F32R, kind="ExternalInput").ap()
    wo_d = nc.dram_tensor("wo", [E, E], F32R, kind="ExternalInput").ap()
    bq_d = nc.dram_tensor("bq", [P, E], F32, kind="ExternalInput").ap()
    bk_d = nc.dram_tensor("bk", [P, KO], F32, kind="ExternalInput").ap()
    bv_d = nc.dram_tensor("bv", [P, E], F32, kind="ExternalInput").ap()
    bo_d = nc.dram_tensor("bo", [P, E], F32, kind="ExternalInput").ap()
    out_d = nc.dram_tensor("out", [BPC, T, E], F32, kind="ExternalOutput").ap()

    add = mybir.AluOpType.add
    mult = mybir.AluOpType.mult
    EXP = mybir.ActivationFunctionType.Exp

    def kslices(ap):  # [E, F] dram -> [128, KO, F] view, partitions = e_in
        return ap.rearrange("(eo ei) f -> ei eo f", ei=P)

    with tile.TileContext(nc) as tc, ExitStack() as ctx:
        consts = ctx.enter_context(tc.tile_pool(name="consts", bufs=1))
        wt = ctx.enter_context(tc.tile_pool(name="wt", bufs=3))
        t1 = ctx.enter_context(tc.tile_pool(name="t1", bufs=1))  # q -> v
        t2 = ctx.enter_context(tc.tile_pool(name="t2", bufs=1))  # kT -> oT
        t3 = ctx.enter_context(tc.tile_pool(name="t3", bufs=1))  # xkT -> aT
        kxm = ctx.enter_context(tc.tile_pool(name="kxm", bufs=2))
        tmp = ctx.enter_context(tc.tile_pool(name="tmp", bufs=3))
        outp = ctx.enter_context(tc.tile_pool(name="outp", bufs=5))
        rec = ctx.enter_context(tc.tile_pool(name="rec", bufs=2))
        pmm = ctx.enter_context(tc.tile_pool(name="pmm", bufs=3, space="PSUM"))
        ptp = ctx.enter_context(tc.tile_pool(name="ptp", bufs=5, space="PSUM"))

        ident_f32 = consts.tile([P, P], F32)
        make_identity(nc, ident_f32)
        ident = consts.tile([P, P], F32R)
        nc.vector.tensor_copy(out=ident[:], in_=ident_f32[:])
        ones_col = consts.tile([P, 2], F32)
        nc.vector.memset(ones_col, 1.0)

        # bias tiles are allocated up front but DMA'd lazily (first use) so
        # the startup DMA queue isn't clogged before the first transposes.
        bq_sb = consts.tile([P, E], F32)
        bk_sb = consts.tile([P, KO], F32)
        bv_sb = consts.tile([P, E], F32)
        bo_sb = consts.tile([P, E], F32)
        _done = set()

        def once(key, fn):
            if key not in _done:
                _done.add(key)
                fn()

        def load_wh(w_d):
            """Weight half-tiles [P, KO, NH] streamed from a shared pool."""
            tiles = []
            for h in range(2):
                wh = wt.tile([P, KO, NH], F32R, tag="wt")
                for ek in range(KO):
                    nc.sync.dma_start(
                        wh[:, ek, :], kslices(w_d)[:, ek, h * NH : (h + 1) * NH]
                    )
                tiles.append(wh)
            return tiles

        def load_transposed_rowtile(x_d, b, r, dst, dst_free_off):
            """DMA rows [r*128, (r+1)*128) of x_d[b] and PE-transpose the
            8 column blocks into dst[:, c, dst_free_off:+128]."""
            t = tmp.tile([P, E], F32R, tag="tmp")
            nc.sync.dma_start(t[:], x_d[b, r * P : (r + 1) * P, :])
            for c in range(KO):
                pt = ptp.tile([P, P], F32R, tag="ptp")
                nc.tensor.transpose(pt[:], t[:, c * P : (c + 1) * P], ident[:])
                nc.any.tensor_copy(
                    out=dst[:, c, dst_free_off : dst_free_off + P], in_=pt[:]
                )

        for b in [b for _ in range(reps) for b in range(BPC)]:
            # ---- q projection: q[t, i] = xq @ Wq.T + bq ----
            wq_h = None
            q_sb = t1.tile([P, KO, E], F32R, tag="t1")
            for m in range(KO):
                xT = kxm.tile([P, KO, P], F32R, tag="kxm")
                load_transposed_rowtile(xq_d, b, m, xT, 0)
                if m == 0:
                    wq_h = load_wh(wq_d)
                    once("bq", lambda: nc.sync.dma_start(bq_sb[:], bq_d))
                for h in range(2):
                    pm = pmm.tile([P, NH], F32, tag="pmm")
                    for ek in range(KO):
                        nc.tensor.matmul(
                            pm[:],
                            xT[:, ek, :],
                            wq_h[h][:, ek, :],
                            start=(ek == 0),
                            stop=(ek == KO - 1),
                        )
                    nc.vector.tensor_tensor(
                        q_sb[:, m, h * NH : (h + 1) * NH],
                        pm[:],
                        bq_sb[:, h * NH : (h + 1) * NH],
                        add,
                    )

            # ---- k projection, transposed: kT[t~, s] = Wk @ xk.T + bk ----
            xkT_sb = t3.tile([P, KO, S], F32R, tag="t3")
            for r in range(KO):
                load_transposed_rowtile(xk_d, b, r, xkT_sb, r * P)
                if r == 0:
                    once("bk", lambda: nc.sync.dma_start(bk_sb[:], bk_d))
            kT_sb = t2.tile([P, KO, S], F32R, tag="t2")
            for m in range(KO):
                wkm = kxm.tile([P, KO, P], F32R, tag="kxm")
                nc.sync.dma_start(wkm[:], kslices(wk_d)[:, :, m * P : (m + 1) * P])
                for h in range(2):
                    pm = pmm.tile([P, NH], F32, tag="pmm")
                    for ek in range(KO):
                        nc.tensor.matmul(
                            pm[:],
                            wkm[:, ek, :],
                            xkT_sb[:, ek, h * NH : (h + 1) * NH],
                            start=(ek == 0),
                            stop=(ek == KO - 1),
                        )
                    nc.vector.tensor_scalar(
                        kT_sb[:, m, h * NH : (h + 1) * NH],
                        pm[:],
                        bk_sb[:, m : m + 1],
                        None,
                        add,
                    )

            # ---- attention logits + exp: aT[s, i] = exp(wT * 1/32) ----
            aT_sb = t3.tile([P, KO, E], F32R, tag="t3")
            for sm in range(KO):
                for h in range(2):
                    pm = pmm.tile([P, NH], F32, tag="pmm")
                    for tk in range(KO):
                        nc.tensor.matmul(
                            pm[:],
                            kT_sb[:, tk, sm * P : (sm + 1) * P],
                            q_sb[:, tk, h * NH : (h + 1) * NH],
                            start=(tk == 0),
                            stop=(tk == KO - 1),
                        )
                    nc.scalar.activation(
                        aT_sb[:, sm, h * NH : (h + 1) * NH],
                        pm[:],
                        EXP,
                        scale=SCALE,
                    )

            # ---- softmax denominators: sums[i] = sum_s aT[s, i] ----
            recip_t = rec.tile([P, KO], F32, tag="rec")
            for im in range(KO):
                ps = ptp.tile([P, 2], F32, tag="ptp")
                for sk in range(KO):
                    nc.tensor.matmul(
                        ps[:],
                        aT_sb[:, sk, im * P : (im + 1) * P].bitcast(F32),
                        ones_col[:],
                        start=(sk == 0),
                        stop=(sk == KO - 1),
                    )
                nc.vector.reciprocal(recip_t[:, im : im + 1], ps[:, 0:1])

            # ---- v projection: v[s, e'] = xv @ Wv.T + bv ----
            wv_h = load_wh(wv_d)
            v_sb = t1.tile([P, KO, E], F32R, tag="t1")
            for m in range(KO):
                xT = kxm.tile([P, KO, P], F32R, tag="kxm")
                load_transposed_rowtile(xv_d, b, m, xT, 0)
                if m == 0:
                    once("bv", lambda: nc.sync.dma_start(bv_sb[:], bv_d))
                for h in range(2):
                    pm = pmm.tile([P, NH], F32, tag="pmm")
                    for ek in range(KO):
                        nc.tensor.matmul(
                            pm[:],
                            xT[:, ek, :],
                            wv_h[h][:, ek, :],
                            start=(ek == 0),
                            stop=(ek == KO - 1),
                        )
                    nc.vector.tensor_tensor(
                        v_sb[:, m, h * NH : (h + 1) * NH],
                        pm[:],
                        bv_sb[:, h * NH : (h + 1) * NH],
                        add,
                    )

            # ---- oT[e', i] = sum_s v[s, e'] aT[s, i]  (unnormalized) ----
            oT_sb = t2.tile([P, KO, E], F32R, tag="t2")
            for em in range(KO):
                for h in range(2):
                    pm = pmm.tile([P, NH], F32, tag="pmm")
                    for sk in range(KO):
                        nc.tensor.matmul(
                            pm[:],
                            v_sb[:, sk, em * P : (em + 1) * P],
                            aT_sb[:, sk, h * NH : (h + 1) * NH],
                            start=(sk == 0),
                            stop=(sk == KO - 1),
                        )
                    nc.any.tensor_copy(
                        out=oT_sb[:, em, h * NH : (h + 1) * NH], in_=pm[:]
                    )

            # ---- out[i, e''] = (oT.T @ Wo.T) * recip[i] + bo ----
            wo_h = load_wh(wo_d)
            once("bo", lambda: nc.sync.dma_start(bo_sb[:], bo_d))
            for im in range(KO):
                for h in range(2):
                    pm = pmm.tile([P, NH], F32, tag="pmm")
                    for ek in range(KO):
                        nc.tensor.matmul(
                            pm[:],
                            oT_sb[:, ek, im * P : (im + 1) * P],
                            wo_h[h][:, ek, :],
                            start=(ek == 0),
                            stop=(ek == KO - 1),
                        )
                    ot = outp.tile([P, NH], F32, tag="outp")
                    nc.vector.tensor_scalar(
                        ot[:], pm[:], recip_t[:, im : im + 1], None, mult
                    )
                    nc.vector.tensor_tensor(
                        ot[:], ot[:], bo_sb[:, h * NH : (h + 1) * NH], add
                    )
                    nc.sync.dma_start(
                        out_d[b, im * P : (im + 1) * P, h * NH : (h + 1) * NH], ot[:]
                    )

    nc.finalize()
    return nc


def _get_nc():
    if "nc" not in _cache:
        _cache["nc"] = _build_nc()
    return _cache["nc"]


def _host_prep(Wq, bq, Wk, bk, Wv, bv, Wo, bo):
    f = np.float32
    return {
        "wq": np.ascontiguousarray(Wq.T, dtype=f),
        "wk": np.ascontiguousarray(Wk.T, dtype=f),
        "wv": np.ascontiguousarray(Wv.T, dtype=f),
        "wo": np.ascontiguousarray(Wo.T, dtype=f),
        "bq": np.ascontiguousarray(np.broadcast_to(bq, (P, E)), dtype=f),
        "bk": np.ascontiguousarray(np.asarray(bk, dtype=f).reshape(KO, P).T),
        "bv": np.ascontiguousarray(np.broadcast_to(bv, (P, E)), dtype=f),
        "bo": np.ascontiguousarray(np.broadcast_to(bo, (P, E)), dtype=f),
    }


def make_in_maps(query, key, value, Wq, bq, Wk, bk, Wv, bv, Wo, bo):
    shared = _host_prep(Wq, bq, Wk, bk, Wv, bv, Wo, bo)
    f = np.float32
    query = np.asarray(query, dtype=f)
    key = np.asarray(key, dtype=f)
    value = np.asarray(value, dtype=f)
    in_maps = []
    for c in range(NCORES):
        sl = slice(c * BPC, (c + 1) * BPC)
        in_maps.append(
            {
                "xq": np.ascontiguousarray(query[sl]),
                "xk": np.ascontiguousarray(key[sl]),
                "xv": np.ascontiguousarray(value[sl]),
                **shared,
            }
        )
    return in_maps


def kernel(query, key, value, Wq, bq, Wk, bk, Wv, bv, Wo, bo):
    from concourse.bass_utils import run_bass_kernel_spmd

    nc = _get_nc()
    in_maps = make_in_maps(query, key, value, Wq, bq, Wk, bk, Wv, bv, Wo, bo)
    res = run_bass_kernel_spmd(nc, in_maps, core_ids=list(range(NCORES)))
    out = np.concatenate([r["out"] for r in res.results], axis=0)
    return out.astype(np.float32)
